# revision 49
# baseline (speedup 1.0000x reference)
"""Trainium2 Bass kernel for nn_ConditionalLayer (moe_routing).

out[i] = x[i] @ W[cond[i]].T + b.sum(0)       x:[8192,1024] W:[16,1024,1024]

Strategy (expert-parallel, host-routed, fp8 hi/lo DoubleRow):
  - Host groups rows by cond value: each of the 8 cores owns 2 of the 16
    experts (slot0 = one of the 8 largest, slot1 = one of the 8 smallest)
    and receives only the rows routed to them.
  - Operands are split hi+lo in fp8 e4m3 at a shared power-of-2 scale:
    v ~= hi + lo with |err| ~ 2^-9 relative, so three DoubleRow matmul
    passes (xh*Wh + xh*Wl + xl*Wh) give ~fp16 accuracy at 0.5 cycles per
    moving row (4x the fp32r MAC rate). Unit 0 of 4 drops W-lo (2 passes)
    to save W bytes; measured end-to-end rel err ~1.5e-2 < 2e-2 gate.
  - W is stationary [128d, 2, 128f]; x is moving [128d, 2, r]; psum gets
    [128f, r] (output transposed; host untransposes while scattering).
    This removes all row padding to 128 multiples (PE cost ~ rows, not
    row-tiles).
  - Compute runs in 4 fc-pair phases x 3 row chunks; each psum group
    accumulates all 11 passes then is evicted (scale 2^-15 + bias, fp16)
    alternately on ACT/DVE. W arrives fc-phase-sliced so the PE streams
    behind the DMA queue; x is issued via Pool SWDGE to keep HWDGE free.
"""

import os
import sys

import numpy as np
import ml_dtypes

_TRN_REPO = "/opt/trn_rl_repo"
if os.path.isdir(_TRN_REPO) and _TRN_REPO not in sys.path:
    sys.path.insert(0, _TRN_REPO)

B, D, C = 8192, 1024, 16
NCORES = 8
P = 128
NU = 4          # 256-wide contraction units
# uneven fc phases: phase 0 is biggest so the PE has work to fill the
# x-streaming window; the last is smallest to shorten the tail
PH_FCS = ((0, 1, 2), (3, 4), (5, 6), (7,))
NQ = len(PH_FCS)
RCHUNK = 512    # max rows per psum group (psum bank = 512 fp32)
E4 = ml_dtypes.float8_e4m3

# per-unit operand copies: unit0 drops x-lo ("wd": W dual, x single —
# measured rel err 0.0137, and x bytes sit on the critical DMA window);
# units 1-3 are full hi+lo both sides ("dd").
UNIT_WLO = (True, True, True, True)
UNIT_XLO = (False, True, True, True)
CU_OF = {}
_cu = 0
for _u in range(NU):
    CU_OF[(_u, 0)] = _cu
    _cu += 1
    if UNIT_WLO[_u]:
        CU_OF[(_u, 1)] = _cu
        _cu += 1
NCU = _cu  # 8
CX_OF = {}
_cx = 0
for _u in range(NU):
    CX_OF[(_u, 0)] = _cx
    _cx += 1
    if UNIT_XLO[_u]:
        CX_OF[(_u, 1)] = _cx
        _cx += 1
NXC = _cx  # 7

# passes per psum group: (unit, x_copy, w_copy), unit-grouped so phase 0
# streams behind the per-unit W+x DMA arrivals; accumulation order commutes.
PASSES = []
for _u in range(NU):
    PASSES.append((_u, 0, 0))            # xh * Wh
    if UNIT_WLO[_u]:
        PASSES.append((_u, 0, 1))        # xh * Wl
    if UNIT_XLO[_u]:
        PASSES.append((_u, 1, 0))        # xl * Wh
NPASS = len(PASSES)  # 11

TRACE = False
LAST_RESULT = None
LAST_NC = None

_nc_cache = {}


def _make_tile_context_cls():
    import concourse.mybir as mybir
    from concourse import tile
    from concourse.vector_clock import ScopedClock

    class TileContextFix(tile.TileContext):
        """This walrus build rejects >1 sync-wait per instruction.  Tile's
        scheduler freely assigns several.  Split the extras onto preceding
        NOPs on the same engine (same-engine program order makes this
        equivalent), and likewise chain the tail drain's waits."""

        _ws_counter = 0

        def _split_multi_waits(self):
            nc = self.nc
            for bb in nc.m.functions[0].blocks:
                insts = list(bb.instructions)
                if not any(
                    i.sync_info
                    and i.sync_info.on_wait
                    and len(i.sync_info.on_wait) > 1
                    for i in insts
                ):
                    continue
                new_seq = []
                for inst in insts:
                    si = inst.sync_info
                    waits = (
                        list(si.on_wait) if (si is not None and si.on_wait) else []
                    )
                    if len(waits) > 1:
                        for w in waits[:-1]:
                            TileContextFix._ws_counter += 1
                            nop = mybir.InstNoOp(
                                name=f"I-waitsplit-{TileContextFix._ws_counter}",
                                engine=inst.engine,
                            )
                            nop.sync_info = mybir.SyncInfo(
                                on_wait=[w], on_update=[]
                            )
                            new_seq.append(nop)
                        inst.sync_info = mybir.SyncInfo(
                            on_wait=[waits[-1]],
                            on_update=list(si.on_update) if si.on_update else [],
                        )
                    new_seq.append(inst)
                bb.instructions[:] = new_seq

        def _drain_and_barrier(self, tick_clock, wait_clock):
            self._split_multi_waits()
            drain_inst = self.nc.sync.drain()
            wait_clock.add_sem_waits(
                drain_inst.ins, ScopedClock({None: tick_clock.global_clock})
            )
            si = drain_inst.ins.sync_info
            waits = list(si.on_wait) if si is not None else []
            if len(waits) > 1:
                drain_inst.ins.sync_info = mybir.SyncInfo(
                    on_wait=waits[:1],
                    on_update=list(si.on_update) if si.on_update else [],
                )
                for w in waits[1:]:
                    extra = self.nc.sync.drain()
                    extra.ins.sync_info = mybir.SyncInfo(on_wait=[w], on_update=[])
            self.nc.all_engine_barrier()
            assert self.sems is not None
            popped = self.nc._tile_sem_poison_stack.pop()
            assert popped is self._sem_poison
            self.nc.clear_and_free_semaphores(list(self.sems.allocated().values()))
            self.nc.all_engine_barrier()

    return TileContextFix


def _chunks_of(M0, M1):
    """Row chunks [(slot, col0, rows)], each <= RCHUNK, per-slot contiguous."""
    chunks = []
    for s, (base, M) in enumerate(((0, M0), (M0, M1))):
        c = 0
        while c < M:
            r = min(RCHUNK, M - c)
            chunks.append((s, base + c, r))
            c += r
    return chunks


def _build(M0, M1, inv_scale):
    key = (M0, M1, inv_scale)
    if key in _nc_cache:
        return _nc_cache[key]

    import concourse.bass as bass
    import concourse.mybir as mybir

    TileContextFix = _make_tile_context_cls()
    DR = mybir.MatmulPerfMode.DoubleRow
    f8 = mybir.dt.float8e4
    f16 = mybir.dt.float16
    f32 = mybir.dt.float32

    M = M0 + M1
    chunks = _chunks_of(M0, M1)
    nchunks = len(chunks)
    assert nchunks <= 4, "psum rotation depth"

    # x split into two column blocks at the expert boundary so all of
    # slot 0's chunks can close before slot 1's x arrives
    BLK0 = M0
    BLK1 = M1

    nc = bass.Bass()
    x8a = nc.declare_dram_parameter("x8a", [NXC, P, 2, BLK0], f8, isOutput=False)
    x8b = nc.declare_dram_parameter("x8b", [NXC, P, 2, BLK1], f8, isOutput=False)
    w8q = [
        nc.declare_dram_parameter(
            f"w8q{q}", [NCU, P, len(fcs), 2, 2, P], f8, isOutput=False
        )
        for q, fcs in enumerate(PH_FCS)
    ]
    bs = nc.declare_dram_parameter("bs", [P, 8], f32, isOutput=False)
    out = nc.declare_dram_parameter("out", [8, P, M], f16, isOutput=True)

    with TileContextFix(nc) as tc:
        with (
            tc.tile_pool(name="xp", bufs=1) as xp,
            tc.tile_pool(name="wp", bufs=1) as wp,
            tc.tile_pool(name="bp", bufs=1) as bp,
            tc.tile_pool(name="pp", bufs=1, space="PSUM") as pp,
            tc.tile_pool(name="op", bufs=4) as op,
        ):
            bias_t = bp.tile([P, 8], f32, tag="bias")

            xt = {}

            def load_x(u, blk, split=False):
                # hi(+lo) together: one DMA per (unit, col-block)
                src, n = (x8a, BLK0) if blk == 0 else (x8b, BLK1)
                ncx = 2 if UNIT_XLO[u] else 1
                cx0 = CX_OF[(u, 0)]
                t = xp.tile(
                    [P, ncx, 2, n], f8, tag=f"x{u}{blk}", name=f"x_{u}_{blk}"
                )
                if split and ncx > 1:
                    nc.sync.dma_start(t[:, 0], src[cx0])
                    nc.sync.dma_start(t[:, 1], src[cx0 + 1])
                else:
                    nc.sync.dma_start(
                        t[:],
                        src[cx0:cx0 + ncx].rearrange("c p i m -> p c i m"),
                    )
                for c in range(ncx):
                    xt[(c, u, blk)] = t[:, c]

            def x_slice(cx, u, c0, r):
                # moving operand [P, 2, r] for rows [c0, c0+r)
                if c0 >= BLK0:
                    return xt[(cx, u, 1)][:, :, c0 - BLK0:c0 - BLK0 + r]
                assert c0 + r <= BLK0
                return xt[(cx, u, 0)][:, :, c0:c0 + r]

            wt = {}

            def load_w(q, u, split_first=False):
                ncv = 2 if UNIT_WLO[u] else 1
                cu0 = CU_OF[(u, 0)]
                njj = len(PH_FCS[q])
                t = wp.tile(
                    [P, ncv, njj, 2, 2, P], f8, tag=f"w{q}{u}",
                    name=f"w_{q}_{u}",
                )
                if split_first:
                    # land the (h, jj0) slice first so the very first
                    # matmul can start ~1.2us earlier
                    nc.sync.dma_start(
                        t[:, 0, 0], w8q[q][cu0, :, 0]
                    )
                    nc.sync.dma_start(
                        t[:, 0, 1:],
                        w8q[q][cu0, :, 1:],
                    )
                    if ncv > 1:
                        nc.sync.dma_start(
                            t[:, 1:],
                            w8q[q][cu0 + 1:cu0 + ncv].rearrange(
                                "cv p jj i e f -> p cv jj i e f"
                            ),
                        )
                else:
                    nc.sync.dma_start(
                        t[:],
                        w8q[q][cu0:cu0 + ncv].rearrange(
                            "cv p jj i e f -> p cv jj i e f"
                        ),
                    )
                wt[(q, u)] = t

            # PE p-state ramp warmup: the cost model runs the tensor engine
            # at ~half clock for its first ~3us of activity.  Start that
            # clock immediately with a zero-cost dummy matmul (plus one per
            # phase-0 W arrival in case idle resets it) so the real matmuls
            # all run at full clock.
            dummy = bp.tile([P, 2], mybir.dt.bfloat16, tag="dummy")
            scr = pp.tile([P, 8], f32, tag="scratch", name="scr", bufs=1)
            nc.vector.memset(dummy[:], 0.0)

            def warm(i, wtile=None):
                if wtile is None:
                    nc.tensor.matmul(
                        scr[:2, i:i + 1],
                        dummy[:, :2],
                        dummy[:, i:i + 1],
                        start=True,
                        stop=True,
                    )
                else:
                    # gated on the W tile's DMA: runs (for ~0 cost) the
                    # moment it lands, keeping the ramp clock alive
                    nc.tensor.matmul(
                        scr[:1, i:i + 1],
                        wtile[:, 0, 0, :, 0, 0:1],
                        wtile[:, 0, 0, :, 0, 0:1],
                        start=True,
                        stop=True,
                        perf_mode=DR,
                    )

            warm(0)

            # stream order = consumption order: (Wq0_u, x_u_blk0) pairs
            # unlock phase 0's first chunk unit by unit, then the second
            # x block (rest of phase 0), then the later phases' W.
            U_ORDER = [u for u in range(1, NU)] + [0]
            for wi, u in enumerate(U_ORDER):
                load_w(0, u)
                load_x(u, 0)
                warm(wi + 1, wt[(0, u)])
                if wi == 0:
                    nc.sync.dma_start(bias_t[:], bs[:])
            for u in U_ORDER:
                load_x(u, 1)
            for q in range(1, NQ):
                for u in range(NU):
                    load_w(q, u)

            # chunk order: phase 0 in x-arrival order (slot 0 first); later
            # phases biggest first so the tail is the smallest chunk's chain
            ci_order = sorted(range(nchunks), key=lambda ci: -chunks[ci][2])
            ci_order0 = sorted(
                range(nchunks), key=lambda ci: (chunks[ci][0], -chunks[ci][2])
            )

            def mk_psum(q, jj, ci):
                # tag per fc-in-phase; bufs 3/2/2 (+1 warmup scratch) fill
                # all 8 banks and pipeline the chunk-major group sequence
                return pp.tile(
                    [P, RCHUNK], f32, tag=f"ps{jj}",
                    name=f"ps_{q}_{jj}_{ci}", bufs=3 if jj == 0 else 2,
                )

            def evict(q, ci, ps, ots):
                s, c0, r = chunks[ci]
                fcs = PH_FCS[q]
                for jj, fc in enumerate(fcs):
                    if (ci + jj) % 2 == 0:
                        nc.scalar.activation(
                            ots[ci][:, jj, :r],
                            ps[jj][:, :r],
                            mybir.ActivationFunctionType.Identity,
                            bias=bias_t[:, fc:fc + 1],
                            scale=inv_scale,
                        )
                    else:
                        nc.vector.tensor_scalar(
                            ots[ci][:, jj, :r],
                            ps[jj][:, :r],
                            inv_scale,
                            bias_t[:, fc:fc + 1],
                            mybir.AluOpType.mult,
                            mybir.AluOpType.add,
                        )
                f0 = fcs[0]
                dst = out[f0:f0 + len(fcs), :, c0:c0 + r].rearrange(
                    "f p m -> p f m"
                )
                if q == NQ - 1:
                    nc.scalar.dma_start(dst, ots[ci][:, :len(fcs), :r])
                else:
                    nc.gpsimd.dma_start(dst, ots[ci][:, :len(fcs), :r])

            for q in range(NQ):
                fcs = PH_FCS[q]
                ots = {}
                for ci, (s, c0, r) in enumerate(chunks):
                    ots[ci] = op.tile(
                        [P, 3, RCHUNK], f16, tag=f"o{ci}", name=f"o_{q}_{ci}"
                    )
                # chunk-major: each chunk's groups accumulate all passes
                # back-to-back, then evict+store immediately.  Phase 0
                # orders passes by DMA arrival (U_ORDER).
                phase_passes = (
                    sorted(PASSES, key=lambda p: U_ORDER.index(p[0]))
                    if q == 0 else PASSES
                )
                for ci in (ci_order0 if q == 0 else ci_order):
                    s, c0, r = chunks[ci]
                    ps = {}
                    for jj in range(len(fcs)):
                        ps[jj] = mk_psum(q, jj, ci)
                    for pi, (u, cx, cw) in enumerate(phase_passes):
                        for jj in range(len(fcs)):
                            cv = CU_OF[(u, cw)] - CU_OF[(u, 0)]
                            nc.tensor.matmul(
                                ps[jj][:, :r],
                                wt[(q, u)][:, cv, jj, :, s, :],
                                x_slice(cx, u, c0, r),
                                start=(pi == 0),
                                stop=(pi == NPASS - 1),
                                perf_mode=DR,
                            )
                    evict(q, ci, ps, ots)

    _nc_cache[key] = nc
    return nc


def _pow2_scale(absmax, fmax=240.0):
    return float(2.0 ** np.floor(np.log2(fmax / max(absmax, 1e-30))))


def kernel(x, cond, W, b):
    from concourse.bass_utils import run_bass_kernel_spmd

    global LAST_RESULT, LAST_NC

    x = np.ascontiguousarray(np.asarray(x, dtype=np.float32))
    cond_i = np.asarray(cond).astype(np.int64)
    W = np.asarray(W, dtype=np.float32)
    b = np.asarray(b, dtype=np.float32)

    counts = np.bincount(cond_i, minlength=C)
    order = np.argsort(-counts, kind="stable")
    slot_experts = (order[:NCORES], order[NCORES:])
    M0 = max(1, int(counts[slot_experts[0]].max()))
    M1 = max(1, int(counts[slot_experts[1]].max()))
    M = M0 + M1

    SX = _pow2_scale(np.abs(x).max())
    SW = _pow2_scale(np.abs(W).max())
    inv_scale = 1.0 / (SX * SW)

    nc = _build(M0, M1, inv_scale)
    LAST_NC = nc

    # --- global quantization (shared across cores) ---
    # xq: [NU, P, 2, B] hi/lo: element (u,p,i,m) = x[m, u*256+i*128+p]
    xsT = np.ascontiguousarray((x.T * SX).reshape(NU, 2, P, B).transpose(0, 2, 1, 3))
    xh = xsT.astype(E4)
    xl = (xsT - xh.astype(np.float32)).astype(E4)

    # Wq per expert: [NU, P, 2, Dout] hi/lo: (u,p,i,f) = W[e][f, u*256+i*128+p]
    Wh_e = np.empty((C, NU, P, 2, D), dtype=E4)
    Wl_e = np.empty((C, NU, P, 2, D), dtype=E4)
    for e in range(C):
        wsT = np.ascontiguousarray(
            (W[e].T * SW).reshape(NU, 2, P, D).transpose(0, 2, 1, 3)
        )
        h = wsT.astype(E4)
        Wh_e[e] = h
        Wl_e[e] = (wsT - h.astype(np.float32)).astype(E4)

    bsum = b.sum(axis=0).astype(np.float32)
    bias_np = np.ascontiguousarray(bsum.reshape(2 * NQ, P).T)  # [P, 8]

    BLK0 = M0
    idx_by_e = [np.nonzero(cond_i == e)[0] for e in range(C)]
    in_maps = []
    placements = []
    for k in range(NCORES):
        eA = int(slot_experts[0][k])
        eB = int(slot_experts[1][k])
        placements.append((k, eA, eB))

        # x: [NXC, P, 2(i), M] — gather routed rows (cols), zero-pad
        x8k = np.zeros((NXC, P, 2, M), dtype=E4)
        for s, (e, col) in enumerate(((eA, 0), (eB, M0))):
            idx = idx_by_e[e]
            for u in range(NU):
                x8k[CX_OF[(u, 0)], :, :, col:col + len(idx)] = xh[u][:, :, idx]
                if UNIT_XLO[u]:
                    x8k[CX_OF[(u, 1)], :, :, col:col + len(idx)] = (
                        xl[u][:, :, idx]
                    )

        # w8q{q}: [NCU, P, 2(i), 2(e), wq]: f-slice per phase
        im = {
            "x8a": np.ascontiguousarray(x8k[:, :, :, :BLK0]),
            "x8b": np.ascontiguousarray(x8k[:, :, :, BLK0:]),
            "bs": bias_np,
        }
        for q, fcs in enumerate(PH_FCS):
            w8kq = np.empty((NCU, P, len(fcs), 2, 2, P), dtype=E4)
            for u in range(NU):
                for cw in range(2 if UNIT_WLO[u] else 1):
                    src = Wh_e if cw == 0 else Wl_e
                    cu = CU_OF[(u, cw)]
                    for ei, e in enumerate((eA, eB)):
                        for jj, fc in enumerate(fcs):
                            w8kq[cu, :, jj, :, ei, :] = (
                                src[e][u][:, :, fc * P:(fc + 1) * P]
                            )
            im[f"w8q{q}"] = w8kq
        in_maps.append(im)

    res = run_bass_kernel_spmd(nc, in_maps, list(range(NCORES)), trace=TRACE)
    LAST_RESULT = res

    out_full = np.empty((B, D), np.float32)
    for k, eA, eB in placements:
        o = res.results[k]["out"]  # [8, P, M] f16
        of = np.asarray(o).reshape(D, M).astype(np.float32)  # f-major [1024, M]
        for e, col in ((eA, 0), (eB, M0)):
            idx = idx_by_e[e]
            out_full[idx] = of[:, col:col + len(idx)].T
    return out_full


if __name__ == "__main__":
    rng = np.random.default_rng(0)
    x = rng.standard_normal((B, D), dtype=np.float32)
    cond = rng.integers(0, C, size=B).astype(np.int64)
    W = (rng.standard_normal((C, D, D), dtype=np.float32) / np.sqrt(D)).astype(
        np.float32
    )
    b = (rng.standard_normal((C, D), dtype=np.float32) * 0.02).astype(np.float32)
    got = kernel(x, cond, W, b)
    want = np.empty((B, D), np.float32)
    for e in range(C):
        idx = np.nonzero(cond == e)[0]
        want[idx] = x[idx] @ W[e].T
    want += b.sum(0)
    denom = np.abs(want).max()
    print("max abs err:", np.abs(got - want).max(), "denom:", denom)
    print("rel err:", np.abs(got - want).max() / denom)


# revision 50
# speedup vs baseline: 1.0062x; 1.0062x over previous
"""Trainium2 Bass kernel for nn_ConditionalLayer (moe_routing).

out[i] = x[i] @ W[cond[i]].T + b.sum(0)       x:[8192,1024] W:[16,1024,1024]

Strategy (expert-parallel, host-routed, fp8 hi/lo DoubleRow):
  - Host groups rows by cond value: each of the 8 cores owns 2 of the 16
    experts (slot0 = one of the 8 largest, slot1 = one of the 8 smallest)
    and receives only the rows routed to them.
  - Operands are split hi+lo in fp8 e4m3 at a shared power-of-2 scale:
    v ~= hi + lo with |err| ~ 2^-9 relative, so three DoubleRow matmul
    passes (xh*Wh + xh*Wl + xl*Wh) give ~fp16 accuracy at 0.5 cycles per
    moving row (4x the fp32r MAC rate). Unit 0 of 4 drops W-lo (2 passes)
    to save W bytes; measured end-to-end rel err ~1.5e-2 < 2e-2 gate.
  - W is stationary [128d, 2, 128f]; x is moving [128d, 2, r]; psum gets
    [128f, r] (output transposed; host untransposes while scattering).
    This removes all row padding to 128 multiples (PE cost ~ rows, not
    row-tiles).
  - Compute runs in 4 fc-pair phases x 3 row chunks; each psum group
    accumulates all 11 passes then is evicted (scale 2^-15 + bias, fp16)
    alternately on ACT/DVE. W arrives fc-phase-sliced so the PE streams
    behind the DMA queue; x is issued via Pool SWDGE to keep HWDGE free.
"""

import os
import sys

import numpy as np
import ml_dtypes

_TRN_REPO = "/opt/trn_rl_repo"
if os.path.isdir(_TRN_REPO) and _TRN_REPO not in sys.path:
    sys.path.insert(0, _TRN_REPO)

B, D, C = 8192, 1024, 16
NCORES = 8
P = 128
NU = 4          # 256-wide contraction units
# uneven fc phases: phase 0 is biggest so the PE has work to fill the
# x-streaming window; the last is smallest to shorten the tail
PH_FCS = ((0, 1, 2), (3, 4), (5, 6), (7,))
NQ = len(PH_FCS)
RCHUNK = 512    # max rows per psum group (psum bank = 512 fp32)
E4 = ml_dtypes.float8_e4m3

# per-unit operand copies: unit0 drops x-lo ("wd": W dual, x single —
# measured rel err 0.0137, and x bytes sit on the critical DMA window);
# units 1-3 are full hi+lo both sides ("dd").
UNIT_WLO = (True, True, True, True)
UNIT_XLO = (False, True, True, True)
CU_OF = {}
_cu = 0
for _u in range(NU):
    CU_OF[(_u, 0)] = _cu
    _cu += 1
    if UNIT_WLO[_u]:
        CU_OF[(_u, 1)] = _cu
        _cu += 1
NCU = _cu  # 8
CX_OF = {}
_cx = 0
for _u in range(NU):
    CX_OF[(_u, 0)] = _cx
    _cx += 1
    if UNIT_XLO[_u]:
        CX_OF[(_u, 1)] = _cx
        _cx += 1
NXC = _cx  # 7

# passes per psum group: (unit, x_copy, w_copy), unit-grouped so phase 0
# streams behind the per-unit W+x DMA arrivals; accumulation order commutes.
PASSES = []
for _u in range(NU):
    PASSES.append((_u, 0, 0))            # xh * Wh
    if UNIT_WLO[_u]:
        PASSES.append((_u, 0, 1))        # xh * Wl
    if UNIT_XLO[_u]:
        PASSES.append((_u, 1, 0))        # xl * Wh
NPASS = len(PASSES)  # 11

TRACE = False
LAST_RESULT = None
LAST_NC = None

_nc_cache = {}


def _make_tile_context_cls():
    import concourse.mybir as mybir
    from concourse import tile
    from concourse.vector_clock import ScopedClock

    class TileContextFix(tile.TileContext):
        """This walrus build rejects >1 sync-wait per instruction.  Tile's
        scheduler freely assigns several.  Split the extras onto preceding
        NOPs on the same engine (same-engine program order makes this
        equivalent), and likewise chain the tail drain's waits."""

        _ws_counter = 0

        def _split_multi_waits(self):
            nc = self.nc
            for bb in nc.m.functions[0].blocks:
                insts = list(bb.instructions)
                if not any(
                    i.sync_info
                    and i.sync_info.on_wait
                    and len(i.sync_info.on_wait) > 1
                    for i in insts
                ):
                    continue
                new_seq = []
                for inst in insts:
                    si = inst.sync_info
                    waits = (
                        list(si.on_wait) if (si is not None and si.on_wait) else []
                    )
                    if len(waits) > 1:
                        for w in waits[:-1]:
                            TileContextFix._ws_counter += 1
                            nop = mybir.InstNoOp(
                                name=f"I-waitsplit-{TileContextFix._ws_counter}",
                                engine=inst.engine,
                            )
                            nop.sync_info = mybir.SyncInfo(
                                on_wait=[w], on_update=[]
                            )
                            new_seq.append(nop)
                        inst.sync_info = mybir.SyncInfo(
                            on_wait=[waits[-1]],
                            on_update=list(si.on_update) if si.on_update else [],
                        )
                    new_seq.append(inst)
                bb.instructions[:] = new_seq

        def _drain_and_barrier(self, tick_clock, wait_clock):
            self._split_multi_waits()
            drain_inst = self.nc.sync.drain()
            wait_clock.add_sem_waits(
                drain_inst.ins, ScopedClock({None: tick_clock.global_clock})
            )
            si = drain_inst.ins.sync_info
            waits = list(si.on_wait) if si is not None else []
            if len(waits) > 1:
                drain_inst.ins.sync_info = mybir.SyncInfo(
                    on_wait=waits[:1],
                    on_update=list(si.on_update) if si.on_update else [],
                )
                for w in waits[1:]:
                    extra = self.nc.sync.drain()
                    extra.ins.sync_info = mybir.SyncInfo(on_wait=[w], on_update=[])
            self.nc.all_engine_barrier()
            assert self.sems is not None
            popped = self.nc._tile_sem_poison_stack.pop()
            assert popped is self._sem_poison
            self.nc.clear_and_free_semaphores(list(self.sems.allocated().values()))
            self.nc.all_engine_barrier()

    return TileContextFix


def _chunks_of(M0, M1):
    """Row chunks [(slot, col0, rows)], each <= RCHUNK, per-slot contiguous."""
    chunks = []
    for s, (base, M) in enumerate(((0, M0), (M0, M1))):
        c = 0
        while c < M:
            r = min(RCHUNK, M - c)
            chunks.append((s, base + c, r))
            c += r
    return chunks


def _build(M0, M1, inv_scale):
    key = (M0, M1, inv_scale)
    if key in _nc_cache:
        return _nc_cache[key]

    import concourse.bass as bass
    import concourse.mybir as mybir

    TileContextFix = _make_tile_context_cls()
    DR = mybir.MatmulPerfMode.DoubleRow
    f8 = mybir.dt.float8e4
    f16 = mybir.dt.float16
    f32 = mybir.dt.float32

    M = M0 + M1
    chunks = _chunks_of(M0, M1)
    nchunks = len(chunks)
    assert nchunks <= 4, "psum rotation depth"

    # x split into two column blocks at the first chunk boundary so the
    # first chunk's psum groups can close before the rest of x arrives
    BLK0 = chunks[0][2]
    BLK1 = M - BLK0

    nc = bass.Bass()
    x8a = nc.declare_dram_parameter("x8a", [NXC, P, 2, BLK0], f8, isOutput=False)
    x8b = nc.declare_dram_parameter("x8b", [NXC, P, 2, BLK1], f8, isOutput=False)
    w8q = [
        nc.declare_dram_parameter(
            f"w8q{q}", [NCU, P, len(fcs), 2, 2, P], f8, isOutput=False
        )
        for q, fcs in enumerate(PH_FCS)
    ]
    bs = nc.declare_dram_parameter("bs", [P, 8], f32, isOutput=False)
    out = nc.declare_dram_parameter("out", [8, P, M], f16, isOutput=True)

    with TileContextFix(nc) as tc:
        with (
            tc.tile_pool(name="xp", bufs=1) as xp,
            tc.tile_pool(name="wp", bufs=1) as wp,
            tc.tile_pool(name="bp", bufs=1) as bp,
            tc.tile_pool(name="pp", bufs=1, space="PSUM") as pp,
            tc.tile_pool(name="op", bufs=4) as op,
        ):
            bias_t = bp.tile([P, 8], f32, tag="bias")

            xt = {}

            def load_x(u, blk, split=False):
                # hi(+lo) together: one DMA per (unit, col-block)
                src, n = (x8a, BLK0) if blk == 0 else (x8b, BLK1)
                ncx = 2 if UNIT_XLO[u] else 1
                cx0 = CX_OF[(u, 0)]
                t = xp.tile(
                    [P, ncx, 2, n], f8, tag=f"x{u}{blk}", name=f"x_{u}_{blk}"
                )
                if split and ncx > 1:
                    nc.sync.dma_start(t[:, 0], src[cx0])
                    nc.sync.dma_start(t[:, 1], src[cx0 + 1])
                else:
                    nc.sync.dma_start(
                        t[:],
                        src[cx0:cx0 + ncx].rearrange("c p i m -> p c i m"),
                    )
                for c in range(ncx):
                    xt[(c, u, blk)] = t[:, c]

            def x_slice(cx, u, c0, r):
                # moving operand [P, 2, r] for rows [c0, c0+r)
                if c0 >= BLK0:
                    return xt[(cx, u, 1)][:, :, c0 - BLK0:c0 - BLK0 + r]
                assert c0 + r <= BLK0
                return xt[(cx, u, 0)][:, :, c0:c0 + r]

            wt = {}

            def load_w(q, u, split_first=False):
                ncv = 2 if UNIT_WLO[u] else 1
                cu0 = CU_OF[(u, 0)]
                njj = len(PH_FCS[q])
                t = wp.tile(
                    [P, ncv, njj, 2, 2, P], f8, tag=f"w{q}{u}",
                    name=f"w_{q}_{u}",
                )
                if split_first:
                    # land the (h, jj0) slice first so the very first
                    # matmul can start ~1.2us earlier
                    nc.sync.dma_start(
                        t[:, 0, 0], w8q[q][cu0, :, 0]
                    )
                    nc.sync.dma_start(
                        t[:, 0, 1:],
                        w8q[q][cu0, :, 1:],
                    )
                    if ncv > 1:
                        nc.sync.dma_start(
                            t[:, 1:],
                            w8q[q][cu0 + 1:cu0 + ncv].rearrange(
                                "cv p jj i e f -> p cv jj i e f"
                            ),
                        )
                else:
                    nc.sync.dma_start(
                        t[:],
                        w8q[q][cu0:cu0 + ncv].rearrange(
                            "cv p jj i e f -> p cv jj i e f"
                        ),
                    )
                wt[(q, u)] = t

            # PE p-state ramp warmup: the cost model runs the tensor engine
            # at ~half clock for its first ~3us of activity.  Start that
            # clock immediately with a zero-cost dummy matmul (plus one per
            # phase-0 W arrival in case idle resets it) so the real matmuls
            # all run at full clock.
            dummy = bp.tile([P, 2], mybir.dt.bfloat16, tag="dummy")
            scr = pp.tile([P, 8], f32, tag="scratch", name="scr", bufs=1)
            nc.vector.memset(dummy[:], 0.0)

            def warm(i, wtile=None):
                if wtile is None:
                    nc.tensor.matmul(
                        scr[:2, i:i + 1],
                        dummy[:, :2],
                        dummy[:, i:i + 1],
                        start=True,
                        stop=True,
                    )
                else:
                    # gated on the W tile's DMA: runs (for ~0 cost) the
                    # moment it lands, keeping the ramp clock alive
                    nc.tensor.matmul(
                        scr[:1, i:i + 1],
                        wtile[:, 0, 0, :, 0, 0:1],
                        wtile[:, 0, 0, :, 0, 0:1],
                        start=True,
                        stop=True,
                        perf_mode=DR,
                    )

            warm(0)

            # stream order = consumption order: (Wq0_u, x_u_blk0) pairs
            # unlock phase 0's first chunk unit by unit, then the second
            # x block (rest of phase 0), then the later phases' W.
            U_ORDER = [u for u in range(1, NU)] + [0]
            for wi, u in enumerate(U_ORDER):
                load_w(0, u)
                load_x(u, 0)
                warm(wi + 1, wt[(0, u)])
                if wi == 0:
                    nc.sync.dma_start(bias_t[:], bs[:])
            for u in U_ORDER:
                load_x(u, 1)
            for q in range(1, NQ):
                for u in range(NU):
                    load_w(q, u)

            # chunk order: phase 0 in x-arrival order (slot 0 first); later
            # phases biggest first so the tail is the smallest chunk's chain
            ci_order = sorted(range(nchunks), key=lambda ci: -chunks[ci][2])
            ci_order0 = sorted(
                range(nchunks), key=lambda ci: (chunks[ci][0], -chunks[ci][2])
            )

            def mk_psum(q, jj, ci):
                # tag per fc-in-phase; bufs 3/2/2 (+1 warmup scratch) fill
                # all 8 banks and pipeline the chunk-major group sequence
                return pp.tile(
                    [P, RCHUNK], f32, tag=f"ps{jj}",
                    name=f"ps_{q}_{jj}_{ci}", bufs=3 if jj == 0 else 2,
                )

            def evict(q, ci, ps, ots):
                s, c0, r = chunks[ci]
                fcs = PH_FCS[q]
                for jj, fc in enumerate(fcs):
                    if (ci + jj) % 2 == 0:
                        nc.scalar.activation(
                            ots[ci][:, jj, :r],
                            ps[jj][:, :r],
                            mybir.ActivationFunctionType.Identity,
                            bias=bias_t[:, fc:fc + 1],
                            scale=inv_scale,
                        )
                    else:
                        nc.vector.tensor_scalar(
                            ots[ci][:, jj, :r],
                            ps[jj][:, :r],
                            inv_scale,
                            bias_t[:, fc:fc + 1],
                            mybir.AluOpType.mult,
                            mybir.AluOpType.add,
                        )
                f0 = fcs[0]
                dst = out[f0:f0 + len(fcs), :, c0:c0 + r].rearrange(
                    "f p m -> p f m"
                )
                if q == NQ - 1:
                    nc.scalar.dma_start(dst, ots[ci][:, :len(fcs), :r])
                else:
                    nc.gpsimd.dma_start(dst, ots[ci][:, :len(fcs), :r])

            for q in range(NQ):
                fcs = PH_FCS[q]
                ots = {}
                for ci, (s, c0, r) in enumerate(chunks):
                    ots[ci] = op.tile(
                        [P, 3, RCHUNK], f16, tag=f"o{ci}", name=f"o_{q}_{ci}"
                    )
                # chunk-major: each chunk's groups accumulate all passes
                # back-to-back, then evict+store immediately.  Phase 0
                # orders passes by DMA arrival (U_ORDER).
                phase_passes = (
                    sorted(PASSES, key=lambda p: U_ORDER.index(p[0]))
                    if q == 0 else PASSES
                )
                for ci in ci_order:
                    s, c0, r = chunks[ci]
                    ps = {}
                    for jj in range(len(fcs)):
                        ps[jj] = mk_psum(q, jj, ci)
                    for pi, (u, cx, cw) in enumerate(phase_passes):
                        for jj in range(len(fcs)):
                            cv = CU_OF[(u, cw)] - CU_OF[(u, 0)]
                            nc.tensor.matmul(
                                ps[jj][:, :r],
                                wt[(q, u)][:, cv, jj, :, s, :],
                                x_slice(cx, u, c0, r),
                                start=(pi == 0),
                                stop=(pi == NPASS - 1),
                                perf_mode=DR,
                            )
                    evict(q, ci, ps, ots)

    _nc_cache[key] = nc
    return nc


def _pow2_scale(absmax, fmax=240.0):
    return float(2.0 ** np.floor(np.log2(fmax / max(absmax, 1e-30))))


def kernel(x, cond, W, b):
    from concourse.bass_utils import run_bass_kernel_spmd

    global LAST_RESULT, LAST_NC

    x = np.ascontiguousarray(np.asarray(x, dtype=np.float32))
    cond_i = np.asarray(cond).astype(np.int64)
    W = np.asarray(W, dtype=np.float32)
    b = np.asarray(b, dtype=np.float32)

    counts = np.bincount(cond_i, minlength=C)
    order = np.argsort(-counts, kind="stable")
    slot_experts = (order[:NCORES], order[NCORES:])
    M0 = max(1, int(counts[slot_experts[0]].max()))
    M1 = max(1, int(counts[slot_experts[1]].max()))
    M = M0 + M1

    SX = _pow2_scale(np.abs(x).max())
    SW = _pow2_scale(np.abs(W).max())
    inv_scale = 1.0 / (SX * SW)

    nc = _build(M0, M1, inv_scale)
    LAST_NC = nc

    # --- global quantization (shared across cores) ---
    # xq: [NU, P, 2, B] hi/lo: element (u,p,i,m) = x[m, u*256+i*128+p]
    xsT = np.ascontiguousarray((x.T * SX).reshape(NU, 2, P, B).transpose(0, 2, 1, 3))
    xh = xsT.astype(E4)
    xl = (xsT - xh.astype(np.float32)).astype(E4)

    # Wq per expert: [NU, P, 2, Dout] hi/lo: (u,p,i,f) = W[e][f, u*256+i*128+p]
    Wh_e = np.empty((C, NU, P, 2, D), dtype=E4)
    Wl_e = np.empty((C, NU, P, 2, D), dtype=E4)
    for e in range(C):
        wsT = np.ascontiguousarray(
            (W[e].T * SW).reshape(NU, 2, P, D).transpose(0, 2, 1, 3)
        )
        h = wsT.astype(E4)
        Wh_e[e] = h
        Wl_e[e] = (wsT - h.astype(np.float32)).astype(E4)

    bsum = b.sum(axis=0).astype(np.float32)
    bias_np = np.ascontiguousarray(bsum.reshape(2 * NQ, P).T)  # [P, 8]

    BLK0 = _chunks_of(M0, M1)[0][2]
    idx_by_e = [np.nonzero(cond_i == e)[0] for e in range(C)]
    in_maps = []
    placements = []
    for k in range(NCORES):
        eA = int(slot_experts[0][k])
        eB = int(slot_experts[1][k])
        placements.append((k, eA, eB))

        # x: [NXC, P, 2(i), M] — gather routed rows (cols), zero-pad
        x8k = np.zeros((NXC, P, 2, M), dtype=E4)
        for s, (e, col) in enumerate(((eA, 0), (eB, M0))):
            idx = idx_by_e[e]
            for u in range(NU):
                x8k[CX_OF[(u, 0)], :, :, col:col + len(idx)] = xh[u][:, :, idx]
                if UNIT_XLO[u]:
                    x8k[CX_OF[(u, 1)], :, :, col:col + len(idx)] = (
                        xl[u][:, :, idx]
                    )

        # w8q{q}: [NCU, P, 2(i), 2(e), wq]: f-slice per phase
        im = {
            "x8a": np.ascontiguousarray(x8k[:, :, :, :BLK0]),
            "x8b": np.ascontiguousarray(x8k[:, :, :, BLK0:]),
            "bs": bias_np,
        }
        for q, fcs in enumerate(PH_FCS):
            w8kq = np.empty((NCU, P, len(fcs), 2, 2, P), dtype=E4)
            for u in range(NU):
                for cw in range(2 if UNIT_WLO[u] else 1):
                    src = Wh_e if cw == 0 else Wl_e
                    cu = CU_OF[(u, cw)]
                    for ei, e in enumerate((eA, eB)):
                        for jj, fc in enumerate(fcs):
                            w8kq[cu, :, jj, :, ei, :] = (
                                src[e][u][:, :, fc * P:(fc + 1) * P]
                            )
            im[f"w8q{q}"] = w8kq
        in_maps.append(im)

    res = run_bass_kernel_spmd(nc, in_maps, list(range(NCORES)), trace=TRACE)
    LAST_RESULT = res

    out_full = np.empty((B, D), np.float32)
    for k, eA, eB in placements:
        o = res.results[k]["out"]  # [8, P, M] f16
        of = np.asarray(o).reshape(D, M).astype(np.float32)  # f-major [1024, M]
        for e, col in ((eA, 0), (eB, M0)):
            idx = idx_by_e[e]
            out_full[idx] = of[:, col:col + len(idx)].T
    return out_full


if __name__ == "__main__":
    rng = np.random.default_rng(0)
    x = rng.standard_normal((B, D), dtype=np.float32)
    cond = rng.integers(0, C, size=B).astype(np.int64)
    W = (rng.standard_normal((C, D, D), dtype=np.float32) / np.sqrt(D)).astype(
        np.float32
    )
    b = (rng.standard_normal((C, D), dtype=np.float32) * 0.02).astype(np.float32)
    got = kernel(x, cond, W, b)
    want = np.empty((B, D), np.float32)
    for e in range(C):
        idx = np.nonzero(cond == e)[0]
        want[idx] = x[idx] @ W[e].T
    want += b.sum(0)
    denom = np.abs(want).max()
    print("max abs err:", np.abs(got - want).max(), "denom:", denom)
    print("rel err:", np.abs(got - want).max() / denom)
